# revision 12
# baseline (speedup 1.0000x reference)
"""Trainium2 Bass kernel for nn_ALNNLayer (ALNN attention-like layer).

Reference computation (per batch b, ref-time k, step l, feature d):
    dist  = |T[b,l,d] - r_k|                      r_k = linspace(0,48,13)
    kern  = exp(-relu(alpha_k) * dist)
    inten = relu(X * kern) = relu(X) * kern       (kern > 0)
    pre   = wt0*X + wt1*DT + wt2*inten + wt3*M + 4*bt
    lat   = relu(pre)
    out[b,k,d] = relu( sum_l wv*lat + 200*bv[k,d] )

Strategy: data-parallel over batch (8 cores x 8 batches). Per core the
SBUF layout is [100 l-partitions, (j=l//100, b, d) free]; weights are
broadcast over b with stride-0 access patterns. Engine split:
  - VectorE: one packed bf16 multiply computes all four products
    (X*wt0 | DT*wt1 | M*wt3 | relu(X)*wt2) in a single [100, 4096] op,
    plus kern-apply (nonzero alpha_k only) and the wv multiply
  - ScalarE: |T-r_k|, exp, and relu fused into the PSUM eviction
  - TensorE: term summation via identity matmuls accumulating in PSUM,
    and the L-reduction via a k-column selector matmul (PSUM outputs
    must start at partition 0, so column k of the selector carries the
    ones and the other 12 output rows accumulate zeros)
k's with relu(alpha_k) == 0 skip dist/exp/kern entirely (kern == 1);
the NEFF is compiled per alpha-sign-pattern, so this stays correct for
any inputs.
"""

import sys

for _p in ("/opt/trn_rl_repo", "/root/.axon_site/_ro/trn_rl_repo"):
    if _p not in sys.path:
        sys.path.append(_p)

import numpy as np
import ml_dtypes

import concourse.bass as bass
import concourse.bacc as bacc
import concourse.tile as tile
from concourse import mybir
from concourse.bass_utils import run_bass_kernel_spmd

B, L, D, K = 64, 200, 64, 13
NCORES = 8
BLOC = B // NCORES  # 8
PRIOR_HOURS = 48.0
REF_TIME = np.linspace(0.0, PRIOR_HOURS, K).astype(np.float32)

LP = 100            # l partitions
LJ = 2              # l super-tiles (l = j*LP + p)
FD = LJ * BLOC * D  # 1024 free elements per partition per (k, f)
NF = 5              # packed product features: X, DT, M, relu(X), ones(*4bt)

F32 = mybir.dt.float32
BF16 = mybir.dt.bfloat16
AX = mybir.AluOpType
AF = mybir.ActivationFunctionType
NPBF = ml_dtypes.bfloat16

# ---- tuning knobs ----
DIST_ENGINE = "act"   # "dve" | "act" | "gps"
Z_ENGINE = "dve"      # "dve" | "gps"
PACK_SPLIT = 1        # 1 = single packed product TT, 2 = two halves


def _bc(ap, nb=BLOC):
    """Insert a stride-0 b dim before the last free dim of an AP."""
    return bass.AP(
        tensor=ap.tensor, offset=ap.offset,
        ap=list(ap.ap[:-1]) + [[0, nb], ap.ap[-1]],
    )


def build_bass(nonzero):
    """nonzero: tuple of bool per k — whether relu(alpha_k) > 0."""
    nc = bacc.Bacc("TRN2", target_bir_lowering=False, debug=False)

    T_d = nc.declare_dram_parameter("T", [BLOC, L, D], F32, isOutput=False)
    X_d = nc.declare_dram_parameter("X", [BLOC, L, D], BF16, isOutput=False)
    DT_d = nc.declare_dram_parameter("DTm", [BLOC, L, D], BF16, isOutput=False)
    M_d = nc.declare_dram_parameter("Mm", [BLOC, L, D], BF16, isOutput=False)
    # per-k weights: [K, LP, 5, LJ, D] products (wt0, wt1, wt3, wt2, 4bt)
    #              | [K, LP, 1, LJ, D] extras (wv)
    W_d = nc.declare_dram_parameter("W", [K, LP, NF + 1, LJ, D], BF16, isOutput=False)
    S_d = nc.declare_dram_parameter("S", [128, 2 * K], F32, isOutput=False)
    BV_d = nc.declare_dram_parameter("BV", [K, D], F32, isOutput=False)  # 200*b_v
    E_d = nc.declare_dram_parameter("ESEL", [128, K * K + 128], BF16, isOutput=False)
    out_d = nc.declare_dram_parameter("out", [BLOC, K, D], F32, isOutput=True)

    from contextlib import ExitStack

    with tile.TileContext(nc) as tc, ExitStack() as ctx:
        const = ctx.enter_context(tc.tile_pool(name="const", bufs=1))
        wpool = ctx.enter_context(tc.tile_pool(name="wpool", bufs=3))
        tmp = ctx.enter_context(tc.tile_pool(name="tmp", bufs=3))
        psum = ctx.enter_context(tc.tile_pool(name="psum", bufs=3, space="PSUM"))
        psum1 = ctx.enter_context(tc.tile_pool(name="psum1", bufs=1, space="PSUM"))

        # ---- resident data, packed [LP, (f, j, b, d)] ----
        Dp = const.tile([LP, NF, LJ, BLOC, D], BF16, tag="Dp")
        for f, dram in ((0, X_d), (1, DT_d), (2, M_d)):
            src = dram[:].rearrange("b (j p) d -> j p b d", j=LJ)
            for j in range(LJ):
                nc.sync.dma_start(out=Dp[:, f, j], in_=src[j])
        Tt = const.tile([LP, LJ, BLOC, D], F32, tag="T")
        srcT = T_d[:].rearrange("b (j p) d -> j p b d", j=LJ)
        for j in range(LJ):
            nc.sync.dma_start(out=Tt[:, j], in_=srcT[j])

        S_sb = const.tile([128, 2 * K], F32)
        nc.sync.dma_start(out=S_sb[:], in_=S_d[:])
        BV_sb = const.tile([K, D], F32)
        nc.sync.dma_start(out=BV_sb[:], in_=BV_d[:])
        E_sb = const.tile([128, K * K + 128], BF16)
        nc.sync.dma_start(out=E_sb[:], in_=E_d[:])
        eye = E_sb[:LP, K * K : K * K + LP]

        # f3 slot <- relu(X); f4 slot <- ones (data for the 4bt term)
        nc.vector.tensor_scalar_max(Dp[:, 3], Dp[:, 0], 0.0)
        nc.vector.memset(Dp[:, 4], 1.0)

        osb = const.tile([K, BLOC, D], F32)
        po = psum1.tile([K, BLOC, D], F32)  # L-sums, one bank, rows = k

        # Software-pipelined emission: for each k, emit ACT abs/exp and
        # the packed product first, then the PE term sums, then k-1's
        # relu/z/selector — so no engine queue head-of-line blocks on a
        # cross-engine latency chain.
        Ws, Sps, Qs = {}, {}, {}

        def stage_front(k):
            w = wpool.tile([LP, NF + 1, LJ, D], BF16, tag="wk")
            nc.sync.dma_start(out=w[:], in_=W_d[k])
            Ws[k] = w
            if nonzero[k]:
                dist = tmp.tile([LP, LJ, BLOC, D], F32, tag="dist")
                if DIST_ENGINE == "act":
                    nc.scalar.activation(
                        dist[:], Tt[:], AF.Abs,
                        bias=S_sb[:LP, K + k : K + k + 1], scale=1.0,
                    )
                else:
                    eng = nc.vector if DIST_ENGINE == "dve" else nc.gpsimd
                    eng.tensor_scalar(
                        dist[:], Tt[:], float(REF_TIME[k]), 0.0,
                        op0=AX.subtract, op1=AX.abs_max,
                    )
                kern = tmp.tile([LP, LJ, BLOC, D], BF16, tag="kern")
                nc.scalar.activation(
                    kern[:], dist[:], AF.Exp, scale=S_sb[:LP, k : k + 1]
                )
            Sp = tmp.tile([LP, NF, LJ, BLOC, D], BF16, tag="Sp")
            nparts = PACK_SPLIT
            fstep = NF // nparts if NF % nparts == 0 else NF
            for f0 in range(0, NF, fstep):
                fn = min(fstep, NF - f0)
                wap = bass.AP(
                    tensor=w[:].tensor,
                    offset=w[:].offset + f0 * LJ * D,
                    ap=[w[:].ap[0], [LJ * D, fn], [D, LJ], [0, BLOC], [1, D]],
                )
                nc.vector.tensor_tensor(
                    Sp[:, f0 : f0 + fn], Dp[:, f0 : f0 + fn], wap, AX.mult
                )
            Sps[k] = Sp
            if nonzero[k]:
                Q = tmp.tile([LP, LJ, BLOC, D], BF16, tag="Q")
                nc.vector.tensor_tensor(Q[:], Sp[:, 3], kern[:], AX.mult)
                Qs[k] = Q

        def stage_mms(k):
            Sp, w = Sps[k], Ws[k]
            qterm = Qs.get(k)
            terms = []
            for f in range(NF):
                if f == 3 and qterm is not None:
                    terms.append(qterm[:])
                else:
                    terms.append(Sp[:, f])
            pre = psum.tile([LP, LJ, BLOC, D], F32, tag="pre")
            for ti, t in enumerate(terms):
                for j in range(LJ):
                    nc.tensor.matmul(
                        pre[:, j], eye, t[:, j],
                        start=(ti == 0), stop=(ti == len(terms) - 1),
                    )
            return pre

        pres = {}

        def stage_back(k):
            pre, w = pres[k], Ws[k]
            lat = tmp.tile([LP, LJ, BLOC, D], BF16, tag="lat")
            nc.scalar.activation(lat[:], pre[:], AF.Relu)
            z = tmp.tile([LP, LJ, BLOC, D], BF16, tag="z")
            zeng = nc.vector if Z_ENGINE == "dve" else nc.gpsimd
            zeng.tensor_tensor(z[:], lat[:], _bc(w[:, NF]), AX.mult)
            for j in range(LJ):
                nc.tensor.matmul(
                    po[:, :, :],
                    E_sb[:LP, k * K : (k + 1) * K],
                    z[:, j],
                    start=(k == 0 and j == 0),
                    stop=(k == K - 1 and j == LJ - 1),
                )

        for k in range(K):
            stage_front(k)
            pres[k] = stage_mms(k)
            if k >= 1:
                stage_back(k - 1)
        stage_back(K - 1)

        # ---- epilogue: out = relu(po + 200*bv) ----
        nc.vector.tensor_tensor(osb[:], po[:], _bc(BV_sb[:]), AX.add)
        nc.vector.tensor_scalar_max(osb[:], osb[:], 0.0)
        nc.sync.dma_start(out=out_d[:].rearrange("b k d -> k b d"), in_=osb[:])

    nc.compile()
    return nc


_NC_CACHE = {}


def _get_nc(nonzero):
    key = tuple(nonzero)
    if key not in _NC_CACHE:
        _NC_CACHE[key] = build_bass(key)
    return _NC_CACHE[key]


def make_in_maps(X, T, M, DT, alpha, w_v, w_t, b_v, b_t):
    X = np.asarray(X, np.float32)
    T = np.asarray(T, np.float32)
    M = np.asarray(M, np.float32)
    DT = np.asarray(DT, np.float32)
    w_t = np.asarray(w_t, np.float32)
    w_v = np.asarray(w_v, np.float32)
    b_t = np.asarray(b_t, np.float32)
    b_v = np.asarray(b_v, np.float32)
    alpha = np.asarray(alpha, np.float32).reshape(K)

    # weight pack: [K, L, 6, D] with f-order (wt0, wt1, wt3, wt2, 4bt, wv)
    W = np.empty((K, L, NF + 1, D), np.float32)
    W[:, :, 0] = w_t[:, :, :, 0]
    W[:, :, 1] = w_t[:, :, :, 1]
    W[:, :, 2] = w_t[:, :, :, 3]
    W[:, :, 3] = w_t[:, :, :, 2]
    W[:, :, 4] = 4.0 * b_t[:, :, :, 0]
    W[:, :, 5] = w_v
    # -> [K, LP, 6, LJ, D] with l = j*LP + p
    W = W.reshape(K, LJ, LP, NF + 1, D).transpose(0, 2, 3, 1, 4)
    W = np.ascontiguousarray(W).astype(NPBF)

    S = np.tile(
        np.concatenate(
            [-np.maximum(alpha.reshape(1, K), 0.0), -REF_TIME.reshape(1, K)], axis=1
        ),
        (128, 1),
    ).astype(np.float32)
    BV = (float(L) * b_v[:, 0, :]).astype(np.float32)
    ESEL = np.zeros((128, K * K + 128), np.float32)
    for k in range(K):
        ESEL[:, k * K + k] = 1.0
    ESEL[:, K * K :] = np.eye(128, dtype=np.float32)
    ESEL = ESEL.astype(NPBF)

    in_maps = []
    for c in range(NCORES):
        b0 = c * BLOC
        in_maps.append(
            {
                "T": np.ascontiguousarray(T[b0 : b0 + BLOC]),
                "X": np.ascontiguousarray(X[b0 : b0 + BLOC]).astype(NPBF),
                "DTm": np.ascontiguousarray(DT[b0 : b0 + BLOC]).astype(NPBF),
                "Mm": np.ascontiguousarray(M[b0 : b0 + BLOC]).astype(NPBF),
                "W": W,
                "S": S,
                "BV": BV,
                "ESEL": ESEL,
            }
        )
    return in_maps, tuple(bool(a > 0) for a in alpha)


def kernel(X, T, M, DT, alpha, w_v, w_t, b_v, b_t):
    in_maps, nonzero = make_in_maps(X, T, M, DT, alpha, w_v, w_t, b_v, b_t)
    nc = _get_nc(nonzero)
    res = run_bass_kernel_spmd(nc, in_maps, core_ids=list(range(NCORES)))
    out = np.concatenate([res.results[c]["out"] for c in range(NCORES)], axis=0)
    return out.astype(np.float32)


# revision 13
# speedup vs baseline: 1.4276x; 1.4276x over previous
"""Trainium2 Bass kernel for nn_ALNNLayer (ALNN attention-like layer).

Reference computation (per batch b, ref-time k, step l, feature d):
    dist  = |T[b,l,d] - r_k|                      r_k = linspace(0,48,13)
    kern  = exp(-relu(alpha_k) * dist)
    inten = relu(X * kern) = relu(X) * kern       (kern > 0)
    pre   = wt0*X + wt1*DT + wt2*inten + wt3*M + 4*bt
    lat   = relu(pre)
    out[b,k,d] = relu( sum_l wv*lat + 200*bv[k,d] )

Strategy: data-parallel over batch (8 cores x 8 batches). Per core the
SBUF layout is [100 l-partitions, (j=l//100, b, d) free]; weights are
broadcast over b with stride-0 access patterns. Engine split:
  - VectorE: one packed bf16 multiply computes all four products
    (X*wt0 | DT*wt1 | M*wt3 | relu(X)*wt2) in a single [100, 4096] op,
    plus kern-apply (nonzero alpha_k only) and the wv multiply
  - ScalarE: |T-r_k|, exp, and relu fused into the PSUM eviction
  - TensorE: term summation via identity matmuls accumulating in PSUM,
    and the L-reduction via a k-column selector matmul (PSUM outputs
    must start at partition 0, so column k of the selector carries the
    ones and the other 12 output rows accumulate zeros)
k's with relu(alpha_k) == 0 skip dist/exp/kern entirely (kern == 1);
the NEFF is compiled per alpha-sign-pattern, so this stays correct for
any inputs.
"""

import sys

for _p in ("/opt/trn_rl_repo", "/root/.axon_site/_ro/trn_rl_repo"):
    if _p not in sys.path:
        sys.path.append(_p)

import numpy as np
import ml_dtypes

import concourse.bass as bass
import concourse.bacc as bacc
import concourse.tile as tile
from concourse import mybir
from concourse.bass_utils import run_bass_kernel_spmd

B, L, D, K = 64, 200, 64, 13
NCORES = 8
BLOC = B // NCORES  # 8
PRIOR_HOURS = 48.0
REF_TIME = np.linspace(0.0, PRIOR_HOURS, K).astype(np.float32)

LP = 100            # l partitions
LJ = 2              # l super-tiles (l = j*LP + p)
FD = LJ * BLOC * D  # 1024 free elements per partition per (k, f)
NF = 4              # packed product features: X, DT, M, relu(X)

F32 = mybir.dt.float32
BF16 = mybir.dt.bfloat16
AX = mybir.AluOpType
AF = mybir.ActivationFunctionType
NPBF = ml_dtypes.bfloat16

# ---- tuning knobs ----
DIST_ENGINE = "act"   # "dve" | "act" | "gps"
Z_ENGINE = "dve"      # "dve" | "gps"
PACK_SPLIT = 1        # 1 = single packed product TT, 2 = two halves


def _bc(ap, nb=BLOC):
    """Insert a stride-0 b dim before the last free dim of an AP."""
    return bass.AP(
        tensor=ap.tensor, offset=ap.offset,
        ap=list(ap.ap[:-1]) + [[0, nb], ap.ap[-1]],
    )


def build_bass(nonzero):
    """nonzero: tuple of bool per k — whether relu(alpha_k) > 0."""
    nc = bacc.Bacc("TRN2", target_bir_lowering=False, debug=False)

    T_d = nc.declare_dram_parameter("T", [BLOC, L, D], F32, isOutput=False)
    X_d = nc.declare_dram_parameter("X", [BLOC, L, D], BF16, isOutput=False)
    DT_d = nc.declare_dram_parameter("DTm", [BLOC, L, D], BF16, isOutput=False)
    M_d = nc.declare_dram_parameter("Mm", [BLOC, L, D], BF16, isOutput=False)
    # per-k weights: [K, LP, 6, LJ, D]: products (wt0, wt1, wt3, wt2) | 4bt | wv
    W_d = nc.declare_dram_parameter("W", [K, LP, NF + 2, LJ, D], BF16, isOutput=False)
    S_d = nc.declare_dram_parameter("S", [128, 2 * K], F32, isOutput=False)
    BV_d = nc.declare_dram_parameter("BV", [K, D], F32, isOutput=False)  # 200*b_v
    E_d = nc.declare_dram_parameter("ESEL", [128, K * K + 128], BF16, isOutput=False)
    out_d = nc.declare_dram_parameter("out", [BLOC, K, D], F32, isOutput=True)

    from contextlib import ExitStack

    with tile.TileContext(nc) as tc, ExitStack() as ctx:
        const = ctx.enter_context(tc.tile_pool(name="const", bufs=1))
        wpool = ctx.enter_context(tc.tile_pool(name="wpool", bufs=5))
        tmp = ctx.enter_context(tc.tile_pool(name="tmp", bufs=4))
        psum = ctx.enter_context(tc.tile_pool(name="psum", bufs=3, space="PSUM"))
        psum1 = ctx.enter_context(tc.tile_pool(name="psum1", bufs=1, space="PSUM"))

        # ---- resident data, packed [LP, (f, j, b, d)] ----
        Dp = const.tile([LP, NF, LJ, BLOC, D], BF16, tag="Dp")
        for f, dram in ((0, X_d), (1, DT_d), (2, M_d)):
            src = dram[:].rearrange("b (j p) d -> j p b d", j=LJ)
            for j in range(LJ):
                nc.sync.dma_start(out=Dp[:, f, j], in_=src[j])
        Tt = const.tile([LP, LJ, BLOC, D], F32, tag="T")
        srcT = T_d[:].rearrange("b (j p) d -> j p b d", j=LJ)
        for j in range(LJ):
            nc.sync.dma_start(out=Tt[:, j], in_=srcT[j])

        S_sb = const.tile([128, 2 * K], F32)
        nc.sync.dma_start(out=S_sb[:], in_=S_d[:])
        BV_sb = const.tile([K, D], F32)
        nc.sync.dma_start(out=BV_sb[:], in_=BV_d[:])
        E_sb = const.tile([128, K * K + 128], BF16)
        nc.sync.dma_start(out=E_sb[:], in_=E_d[:])
        eye = E_sb[:LP, K * K : K * K + LP]

        # f3 slot <- relu(X)
        nc.vector.tensor_scalar_max(Dp[:, 3], Dp[:, 0], 0.0)

        osb = const.tile([K, BLOC, D], F32)
        po = psum1.tile([K, BLOC, D], F32)  # L-sums, one bank, rows = k

        # Software-pipelined emission: for each k, emit ACT abs/exp and
        # the packed product first, then the PE term sums, then k-1's
        # relu/z/selector — so no engine queue head-of-line blocks on a
        # cross-engine latency chain.
        Ws, Sps, Qs = {}, {}, {}

        def stage_front(k):
            w = wpool.tile([LP, NF + 2, LJ, D], BF16, tag="wk")
            nc.sync.dma_start(out=w[:], in_=W_d[k])
            Ws[k] = w
            if nonzero[k]:
                dist = tmp.tile([LP, LJ, BLOC, D], F32, tag="dist")
                if DIST_ENGINE == "act":
                    nc.scalar.activation(
                        dist[:], Tt[:], AF.Abs,
                        bias=S_sb[:LP, K + k : K + k + 1], scale=1.0,
                    )
                else:
                    eng = nc.vector if DIST_ENGINE == "dve" else nc.gpsimd
                    eng.tensor_scalar(
                        dist[:], Tt[:], float(REF_TIME[k]), 0.0,
                        op0=AX.subtract, op1=AX.abs_max,
                    )
                kern = tmp.tile([LP, LJ, BLOC, D], BF16, tag="kern")
                nc.scalar.activation(
                    kern[:], dist[:], AF.Exp, scale=S_sb[:LP, k : k + 1]
                )
            Sp = tmp.tile([LP, NF, LJ, BLOC, D], BF16, tag="Sp")
            nparts = PACK_SPLIT
            fstep = NF // nparts if NF % nparts == 0 else NF
            for f0 in range(0, NF, fstep):
                fn = min(fstep, NF - f0)
                wap = bass.AP(
                    tensor=w[:].tensor,
                    offset=w[:].offset + f0 * LJ * D,
                    ap=[w[:].ap[0], [LJ * D, fn], [D, LJ], [0, BLOC], [1, D]],
                )
                nc.vector.tensor_tensor(
                    Sp[:, f0 : f0 + fn], Dp[:, f0 : f0 + fn], wap, AX.mult
                )
            Sps[k] = Sp
            if nonzero[k]:
                Q = tmp.tile([LP, LJ, BLOC, D], BF16, tag="Q")
                nc.vector.tensor_tensor(Q[:], Sp[:, 3], kern[:], AX.mult)
                Qs[k] = Q

        def stage_mms(k):
            Sp, w = Sps[k], Ws[k]
            qterm = Qs.get(k)
            # Q (ACT-chain dependent) goes last so it can't stall the
            # earlier matmuls in the PE queue.
            terms = [Sp[:, 0], Sp[:, 1], Sp[:, 2]]
            terms.append(_bc(w[:, NF]))  # 4bt broadcast over b
            terms.append(qterm[:] if qterm is not None else Sp[:, 3])
            pre = psum.tile([LP, LJ, BLOC, D], F32, tag="pre")
            for ti, t in enumerate(terms):
                for j in range(LJ):
                    nc.tensor.matmul(
                        pre[:, j], eye, t[:, j],
                        start=(ti == 0), stop=(ti == len(terms) - 1),
                    )
            return pre

        pres = {}

        def stage_back(k):
            pre, w = pres[k], Ws[k]
            lat = tmp.tile([LP, LJ, BLOC, D], BF16, tag="lat")
            nc.scalar.activation(lat[:], pre[:], AF.Relu)
            z = tmp.tile([LP, LJ, BLOC, D], BF16, tag="z")
            zeng = nc.vector if Z_ENGINE == "dve" else nc.gpsimd
            zeng.tensor_tensor(z[:], lat[:], _bc(w[:, NF + 1]), AX.mult)
            for j in range(LJ):
                nc.tensor.matmul(
                    po[:, :, :],
                    E_sb[:LP, k * K : (k + 1) * K],
                    z[:, j],
                    start=(k == 0 and j == 0),
                    stop=(k == K - 1 and j == LJ - 1),
                )

        stage_front(0)
        stage_front(1)
        for k in range(K):
            if k + 2 < K:
                stage_front(k + 2)
            pres[k] = stage_mms(k)
            if k >= 1:
                stage_back(k - 1)
        stage_back(K - 1)

        # ---- epilogue: out = relu(po + 200*bv) ----
        nc.vector.tensor_tensor(osb[:], po[:], _bc(BV_sb[:]), AX.add)
        nc.vector.tensor_scalar_max(osb[:], osb[:], 0.0)
        nc.sync.dma_start(out=out_d[:].rearrange("b k d -> k b d"), in_=osb[:])

    nc.compile()
    return nc


_NC_CACHE = {}


def _get_nc(nonzero):
    key = tuple(nonzero)
    if key not in _NC_CACHE:
        _NC_CACHE[key] = build_bass(key)
    return _NC_CACHE[key]


def make_in_maps(X, T, M, DT, alpha, w_v, w_t, b_v, b_t):
    X = np.asarray(X, np.float32)
    T = np.asarray(T, np.float32)
    M = np.asarray(M, np.float32)
    DT = np.asarray(DT, np.float32)
    w_t = np.asarray(w_t, np.float32)
    w_v = np.asarray(w_v, np.float32)
    b_t = np.asarray(b_t, np.float32)
    b_v = np.asarray(b_v, np.float32)
    alpha = np.asarray(alpha, np.float32).reshape(K)

    # weight pack: [K, L, 6, D] with f-order (wt0, wt1, wt3, wt2, 4bt, wv)
    W = np.empty((K, L, NF + 2, D), np.float32)
    W[:, :, 0] = w_t[:, :, :, 0]
    W[:, :, 1] = w_t[:, :, :, 1]
    W[:, :, 2] = w_t[:, :, :, 3]
    W[:, :, 3] = w_t[:, :, :, 2]
    W[:, :, 4] = 4.0 * b_t[:, :, :, 0]
    W[:, :, 5] = w_v
    # -> [K, LP, 6, LJ, D] with l = j*LP + p
    W = W.reshape(K, LJ, LP, NF + 2, D).transpose(0, 2, 3, 1, 4)
    W = np.ascontiguousarray(W).astype(NPBF)

    S = np.tile(
        np.concatenate(
            [-np.maximum(alpha.reshape(1, K), 0.0), -REF_TIME.reshape(1, K)], axis=1
        ),
        (128, 1),
    ).astype(np.float32)
    BV = (float(L) * b_v[:, 0, :]).astype(np.float32)
    ESEL = np.zeros((128, K * K + 128), np.float32)
    for k in range(K):
        ESEL[:, k * K + k] = 1.0
    ESEL[:, K * K :] = np.eye(128, dtype=np.float32)
    ESEL = ESEL.astype(NPBF)

    in_maps = []
    for c in range(NCORES):
        b0 = c * BLOC
        in_maps.append(
            {
                "T": np.ascontiguousarray(T[b0 : b0 + BLOC]),
                "X": np.ascontiguousarray(X[b0 : b0 + BLOC]).astype(NPBF),
                "DTm": np.ascontiguousarray(DT[b0 : b0 + BLOC]).astype(NPBF),
                "Mm": np.ascontiguousarray(M[b0 : b0 + BLOC]).astype(NPBF),
                "W": W,
                "S": S,
                "BV": BV,
                "ESEL": ESEL,
            }
        )
    return in_maps, tuple(bool(a > 0) for a in alpha)


def kernel(X, T, M, DT, alpha, w_v, w_t, b_v, b_t):
    in_maps, nonzero = make_in_maps(X, T, M, DT, alpha, w_v, w_t, b_v, b_t)
    nc = _get_nc(nonzero)
    res = run_bass_kernel_spmd(nc, in_maps, core_ids=list(range(NCORES)))
    out = np.concatenate([res.results[c]["out"] for c in range(NCORES)], axis=0)
    return out.astype(np.float32)
